# revision 10
# baseline (speedup 1.0000x reference)
"""Bass/Tile kernel for nn_Decoder (Bahdanau-attention LSTM decoder).

Per core (data-parallel over batch, 8 cores x 64 batch):
  - enc_proj^T = w1_enc @ enc^T precomputed once, resident in SBUF (bf16),
    layout (e'-chunk 128, b*t free).
  - encfc[b,t] = enc[b,t,:] . fc_w  precomputed once: during the scan the
    attention context is only needed through y_tilde = fc.ctx = S1/S0 where
    S0 = sum_t exp(score), S1 = sum_t exp(score)*encfc.  The full context
    vector is materialized once, after the last step.
  - scan over 256 steps: q = [h;c] @ w1_hc^T (PE) -> +q per-batch via DVE
    tensor_scalar -> one big ACT tanh per (group, e-half) -> scores via
    LDW+1-col matmuls into (t, b) PSUM -> exp (no max-sub; scores bounded)
    -> S0/S1 via ones-matmul -> y_tilde -> LSTM gates (PE) -> gate
    activations as tanh(x/2) (keeps ACT on one table set) -> DVE elementwise.
  - batch split into 2 groups of 32 so ACT/PE/DVE pipeline across groups.
"""

import numpy as np
import ml_dtypes
from contextlib import ExitStack

import concourse.bacc as bacc
import concourse.bass as bass
import concourse.mybir as mybir
import concourse.tile as tile
from concourse.bass import ds
from concourse.bass_utils import run_bass_kernel_spmd
from concourse.masks import make_identity

B, T, E, D = 512, 256, 256, 256
NCORES = 8
BL = B // NCORES          # 64 batch per core
NG = 2                    # pipeline groups per core
GB = BL // NG             # 32 batch per group
NSTEP = T                 # 256 scan steps

F32 = mybir.dt.float32
BF16 = mybir.dt.bfloat16
AF = mybir.ActivationFunctionType

_BUILT = None


def _emit_step(nc, tc, w, st, pools, yoff, persist=None):
    """Emit one scan step (both groups). yoff: yterm free-dim offset (int or
    loop scalar). persist: dict of persistent tiles for the final step."""
    for g in range(NG):
        # ---- q = [h;c] @ w1_hc^T (+b1 in drain) -> (e' 2x128 packed, b) ----
        q_ps = pools["qps"].tile([128, 2 * GB], F32)
        rhs_chunks = [st["h2b"][g][:, 0:GB], st["h2b"][g][:, GB:2 * GB],
                      st["cb"][g][:, 0:GB], st["cb"][g][:, GB:2 * GB]]
        for eh in range(2):
            for pc in range(4):
                nc.tensor.matmul(
                    q_ps[:, eh * GB:(eh + 1) * GB],
                    w["w1hcT"][pc][:, eh * 128:(eh + 1) * 128],
                    rhs_chunks[pc],
                    start=(pc == 0), stop=(pc == 3))
        q_sb = pools["qsb"].tile([128, 2 * GB], F32)
        for eh in range(2):
            nc.vector.tensor_scalar_add(
                q_sb[:, eh * GB:(eh + 1) * GB],
                q_ps[:, eh * GB:(eh + 1) * GB],
                w["b1c"][:, eh:eh + 1])

        # ---- arg = encp + q ; tanh (in-place) ----
        A = []
        for eh in range(2):
            a = pools["arg"].tile([128, GB * T], BF16)
            for bl in range(GB):
                nc.vector.tensor_scalar_add(
                    a[:, bl * T:(bl + 1) * T],
                    w["encp"][eh][:, (g * GB + bl) * T:(g * GB + bl + 1) * T],
                    q_sb[:, eh * GB + bl:eh * GB + bl + 1])
            nc.scalar.activation(a[:], a[:], AF.Tanh)
            A.append(a)

        # ---- scores: (t 2x128 packed, b) PSUM ----
        sc_ps = pools["scps"].tile([128, 2 * GB], F32)
        for bl in range(GB):
            for th in range(2):
                for eh in range(2):
                    nc.tensor.matmul(
                        sc_ps[:, th * GB + bl:th * GB + bl + 1],
                        A[eh][:, bl * T + th * 128:bl * T + (th + 1) * 128],
                        w["w2c"][:, eh:eh + 1],
                        start=(eh == 0), stop=(eh == 1))

        # ---- exp / S0 / S1 ----
        if persist is not None:
            exp_g = persist["exp"][g]
        else:
            exp_g = pools["exp"].tile([128, 2 * GB], BF16)
        nc.scalar.activation(exp_g[:], sc_ps[:], AF.Exp)
        ef_g = pools["ef"].tile([128, 2 * GB], BF16)
        for th in range(2):
            nc.vector.tensor_mul(
                ef_g[:, th * GB:(th + 1) * GB],
                exp_g[:, th * GB:(th + 1) * GB],
                w["encfcT"][th][:, g * GB:(g + 1) * GB])
        s_ps = pools["sps"].tile([1, 2 * GB], F32)
        for th in range(2):
            nc.tensor.matmul(s_ps[0:1, 0:GB], w["ones"],
                             exp_g[:, th * GB:(th + 1) * GB],
                             start=(th == 0), stop=(th == 1))
        for th in range(2):
            nc.tensor.matmul(s_ps[0:1, GB:2 * GB], w["ones"],
                             ef_g[:, th * GB:(th + 1) * GB],
                             start=(th == 0), stop=(th == 1))

        # ---- y_tilde = S1/S0 + yterm[s] ----
        r_g = pools["rg"].tile([1, GB], F32)
        nc.vector.reciprocal(r_g[:], s_ps[0:1, 0:GB])
        yt_g = pools["ytg"].tile([1, GB], F32)
        nc.vector.tensor_mul(yt_g[:], s_ps[0:1, GB:2 * GB], r_g[:])
        yrow = pools["yrow"].tile([1, GB], F32)
        if isinstance(yoff, int):
            ysl = w["yterm"][0:1, yoff + g * GB: yoff + (g + 1) * GB]
        else:
            ysl = w["yterm"][0:1, ds(yoff + g * GB, GB)]
        nc.sync.dma_start(out=yrow[:], in_=ysl)
        nc.vector.tensor_add(yt_g[:], yt_g[:], yrow[:])
        nc.vector.tensor_copy(st["yt1"][g][0:1, :], yt_g[:])
        if persist is not None:
            nc.vector.tensor_copy(persist["s0"][g][:], s_ps[0:1, 0:GB])

        # ---- gates (PE) -> one ACT tanh(x/2) ----
        g_ps = pools["gps"].tile([128, 8 * GB], F32)
        for m in range(8):
            o = g_ps[:, m * GB:(m + 1) * GB]
            nc.tensor.matmul(o, w["whhT"][0][:, m * 128:(m + 1) * 128],
                             st["h2b"][g][:, 0:GB], start=True, stop=False)
            nc.tensor.matmul(o, w["whhT"][1][:, m * 128:(m + 1) * 128],
                             st["h2b"][g][:, GB:2 * GB], start=False, stop=False)
            nc.tensor.matmul(o, w["wihbT"][:, m * 128:(m + 1) * 128],
                             st["yt1"][g][:], start=False, stop=True)
        th_all = pools["thall"].tile([128, 8 * GB], F32)
        nc.scalar.activation(th_all[:], g_ps[:], AF.Tanh, scale=0.5)

        # ---- LSTM elementwise ----
        # c_new = 0.5*(th_f*c + c + th_i*th_g + th_g); h2 = (th_o+1)*tanh(c_new)
        thi = th_all[:, 0:2 * GB]
        thf = th_all[:, 2 * GB:4 * GB]
        thg = th_all[:, 4 * GB:6 * GB]
        tho = th_all[:, 6 * GB:8 * GB]
        c_g = st["c"][g]
        t1 = pools["lt1"].tile([128, 2 * GB], F32)
        nc.vector.tensor_mul(t1[:], thf, c_g[:])
        nc.vector.tensor_add(t1[:], t1[:], c_g[:])
        t2 = pools["lt2"].tile([128, 2 * GB], F32)
        nc.vector.tensor_mul(t2[:], thi, thg)
        nc.vector.tensor_add(t2[:], t2[:], thg)
        nc.vector.tensor_add(t1[:], t1[:], t2[:])
        nc.vector.tensor_scalar_mul(c_g[:], t1[:], 0.5)
        thc = pools["thc"].tile([128, 2 * GB], F32)
        nc.scalar.activation(thc[:], c_g[:], AF.Tanh)
        t3 = pools["lt3"].tile([128, 2 * GB], F32)
        nc.vector.tensor_scalar_add(t3[:], tho, 1.0)
        if persist is not None:
            h2f = persist["h2f"][g]
            nc.vector.tensor_mul(h2f[:], t3[:], thc[:])
            nc.vector.tensor_copy(st["h2b"][g][:], h2f[:])
        else:
            h2f = pools["h2f"].tile([128, 2 * GB], F32)
            nc.vector.tensor_mul(h2f[:], t3[:], thc[:])
            nc.vector.tensor_copy(st["h2b"][g][:], h2f[:])
        nc.vector.tensor_copy(st["cb"][g][:], c_g[:])


def build_nc():
    nc = bacc.Bacc("TRN2", target_bir_lowering=False, debug=False,
                   num_devices=NCORES)
    enc = nc.dram_tensor("enc", [BL * T, E], F32, kind="ExternalInput").ap()
    d_yterm = nc.dram_tensor("yterm", [1, T * BL], F32, kind="ExternalInput").ap()
    d_w1eT = nc.dram_tensor("w1eT", [E, E], BF16, kind="ExternalInput").ap()
    d_w1hcT = nc.dram_tensor("w1hcT", [2 * D, E], BF16, kind="ExternalInput").ap()
    d_w2c = nc.dram_tensor("w2c", [128, 2], BF16, kind="ExternalInput").ap()
    d_b1c = nc.dram_tensor("b1c", [128, 2], F32, kind="ExternalInput").ap()
    d_fcwc = nc.dram_tensor("fcwc", [128, 2], BF16, kind="ExternalInput").ap()
    d_whhT = nc.dram_tensor("whhT", [D, 4 * D], BF16, kind="ExternalInput").ap()
    d_wihbT = nc.dram_tensor("wihbT", [128, 4 * D], BF16, kind="ExternalInput").ap()
    d_h2o = nc.dram_tensor("h2o", [NG, 128, 2 * GB], F32, kind="ExternalOutput").ap()
    d_ctxo = nc.dram_tensor("ctxo", [2, 128, BL], F32, kind="ExternalOutput").ap()
    d_s0o = nc.dram_tensor("s0o", [NG, 1, GB], F32, kind="ExternalOutput").ap()

    with tile.TileContext(nc) as tc, ExitStack() as top:
        singles = top.enter_context(tc.tile_pool(name="singles", bufs=1))

        w = {}
        w["w1hcT"] = [singles.tile([128, E], BF16, tag=f"w1hcT{i}", name=f"w1hcT{i}") for i in range(4)]
        for pc in range(4):
            nc.sync.dma_start(out=w["w1hcT"][pc][:],
                              in_=d_w1hcT[pc * 128:(pc + 1) * 128, :])
        w1eT_sb = [singles.tile([128, E], BF16, tag=f"w1eT{i}", name=f"w1eT{i}") for i in range(2)]
        for eh in range(2):
            nc.sync.dma_start(out=w1eT_sb[eh][:],
                              in_=d_w1eT[eh * 128:(eh + 1) * 128, :])
        for nm, dt_, src in (("w2c", BF16, d_w2c), ("b1c", F32, d_b1c),
                             ("fcwc", BF16, d_fcwc)):
            w[nm] = singles.tile([128, 2], dt_, tag=nm, name=nm)
            nc.sync.dma_start(out=w[nm][:], in_=src[:])
        w["whhT"] = [singles.tile([128, 4 * D], BF16, tag=f"whhT{i}", name=f"whhT{i}") for i in range(2)]
        for dc in range(2):
            nc.sync.dma_start(out=w["whhT"][dc][:],
                              in_=d_whhT[dc * 128:(dc + 1) * 128, :])
        w["wihbT"] = singles.tile([128, 4 * D], BF16, tag="wihbT", name="wihbT")
        nc.sync.dma_start(out=w["wihbT"][:], in_=d_wihbT[:])
        w["yterm"] = d_yterm
        ones = singles.tile([128, 1], BF16, tag="ones", name="ones")
        nc.vector.memset(ones[:], 1.0)
        w["ones"] = ones[:]
        ident = singles.tile([128, 128], F32, tag="ident", name="ident")
        make_identity(nc, ident[:])

        w["encp"] = [singles.tile([128, BL * T], BF16, tag=f"encp{i}", name=f"encp{i}")
                     for i in range(2)]
        w["encfcT"] = [singles.tile([128, BL], BF16, tag=f"encfcT{i}", name=f"encfcT{i}")
                       for i in range(2)]

        # ---------------- precompute phase ----------------
        with ExitStack() as ph:
            ph_pool = ph.enter_context(tc.tile_pool(name="ph", bufs=1))
            encT = [ph_pool.tile([128, BL * T], BF16, tag=f"encT{i}", name=f"encT{i}")
                    for i in range(2)]
            raws = ph.enter_context(tc.tile_pool(name="raws", bufs=4))
            tp_ps = ph.enter_context(
                tc.tile_pool(name="tpps", bufs=4, space="PSUM"))
            mm_ps = ph.enter_context(
                tc.tile_pool(name="mmps", bufs=3, space="PSUM"))
            # load + transpose enc -> encT (e, b*t)
            for i in range(BL * T // 128):
                r = raws.tile([128, E], F32)
                nc.sync.dma_start(out=r[:], in_=enc[i * 128:(i + 1) * 128, :])
                for eh in range(2):
                    pst = tp_ps.tile([128, 128], F32)
                    nc.tensor.transpose(pst[:], r[:, eh * 128:(eh + 1) * 128],
                                        ident[:])
                    if (i + eh) % 2 == 0:
                        nc.vector.tensor_copy(
                            encT[eh][:, i * 128:(i + 1) * 128], pst[:])
                    else:
                        nc.scalar.copy(
                            encT[eh][:, i * 128:(i + 1) * 128], pst[:])
            # enc_proj^T = w1_enc @ enc^T
            for ep in range(2):
                for jc in range(BL * T // 512):
                    ps = mm_ps.tile([128, 512], F32)
                    for eh in range(2):
                        nc.tensor.matmul(
                            ps[:], w1eT_sb[eh][:, ep * 128:(ep + 1) * 128],
                            encT[eh][:, jc * 512:(jc + 1) * 512],
                            start=(eh == 0), stop=(eh == 1))
                    if jc % 2 == 0:
                        nc.vector.tensor_copy(
                            w["encp"][ep][:, jc * 512:(jc + 1) * 512], ps[:])
                    else:
                        nc.scalar.copy(
                            w["encp"][ep][:, jc * 512:(jc + 1) * 512], ps[:])
            # encfc^T (t-chunk, b)
            for th in range(2):
                efps = mm_ps.tile([128, BL], F32, tag="efps", bufs=1)
                for b in range(BL):
                    for eh in range(2):
                        nc.tensor.matmul(
                            efps[:, b:b + 1],
                            encT[eh][:, b * T + th * 128:b * T + (th + 1) * 128],
                            w["fcwc"][:, eh:eh + 1],
                            start=(eh == 0), stop=(eh == 1))
                nc.vector.tensor_copy(w["encfcT"][th][:], efps[:])

        # ---------------- state ----------------
        st = {"c": [], "h2b": [], "cb": [], "yt1": []}
        for g in range(NG):
            cg = singles.tile([128, 2 * GB], F32, tag=f"c{g}", name=f"c{g}")
            nc.vector.memset(cg[:], 0.0)
            st["c"].append(cg)
            hb = singles.tile([128, 2 * GB], BF16, tag=f"h2b{g}", name=f"h2b{g}")
            nc.vector.memset(hb[:], 0.0)
            st["h2b"].append(hb)
            cb = singles.tile([128, 2 * GB], BF16, tag=f"cb{g}", name=f"cb{g}")
            nc.vector.memset(cb[:], 0.0)
            st["cb"].append(cb)
            y1 = singles.tile([128, GB], BF16, tag=f"yt1{g}", name=f"yt1{g}")
            # row0 is overwritten with y_tilde each step; row1 pairs with the
            # bias row of wihbT; rows 2+ hit zero weight rows (host-zeroed).
            nc.vector.memset(y1[:], 1.0)
            st["yt1"].append(y1)

        persist = {
            "exp": [singles.tile([128, 2 * GB], BF16, tag=f"expP{g}", name=f"expP{g}")
                    for g in range(NG)],
            "h2f": [singles.tile([128, 2 * GB], F32, tag=f"h2fP{g}", name=f"h2fP{g}")
                    for g in range(NG)],
            "s0": [singles.tile([1, GB], F32, tag=f"s0P{g}", name=f"s0P{g}")
                   for g in range(NG)],
        }

        # ---------------- scan ----------------
        with ExitStack() as sc:
            pools = {}
            for nm, bufs, space in (
                    ("qps", 2, "PSUM"), ("scps", 2, "PSUM"),
                    ("sps", 2, "PSUM"), ("gps", 2, "PSUM"),
                    ("arg", 3, "SBUF"), ("qsb", 2, "SBUF"),
                    ("exp", 2, "SBUF"), ("ef", 2, "SBUF"),
                    ("rg", 2, "SBUF"), ("ytg", 2, "SBUF"),
                    ("thall", 2, "SBUF"), ("lt1", 2, "SBUF"),
                    ("lt2", 2, "SBUF"), ("lt3", 2, "SBUF"),
                    ("thc", 2, "SBUF"), ("h2f", 2, "SBUF"),
                    ("yrow", 4, "SBUF")):
                pools[nm] = sc.enter_context(
                    tc.tile_pool(name=nm, bufs=bufs, space=space))

            with tc.For_i(0, (NSTEP - 1) * BL, BL,
                          hint_engines=(mybir.EngineType.PE,)) as iv:
                _emit_step(nc, tc, w, st, pools, iv)
            _emit_step(nc, tc, w, st, pools, (NSTEP - 1) * BL, persist=persist)

        # ---------------- final context + outputs ----------------
        with ExitStack() as fin:
            raws = fin.enter_context(tc.tile_pool(name="fraw", bufs=4))
            bfs = fin.enter_context(tc.tile_pool(name="fbf", bufs=6))
            cps = fin.enter_context(tc.tile_pool(name="cps", bufs=2,
                                                 space="PSUM"))
            ctx_ps = [cps.tile([128, BL], F32, tag=f"ctxps{e}", name=f"ctxps{e}")
                      for e in range(2)]
            for i in range(128):          # i = b*2 + th
                b, th = i // 2, i % 2
                g, bl = b // GB, b % GB
                r = raws.tile([128, E], F32)
                nc.sync.dma_start(out=r[:],
                                  in_=enc[b * T + th * 128:b * T + (th + 1) * 128, :])
                rb = bfs.tile([128, E], BF16)
                nc.vector.tensor_copy(rb[:], r[:])
                for eh in range(2):
                    nc.tensor.matmul(
                        ctx_ps[eh][:, b:b + 1],
                        rb[:, eh * 128:(eh + 1) * 128],
                        persist["exp"][g][:, th * GB + bl:th * GB + bl + 1],
                        start=(th == 0), stop=(th == 1))
            for eh in range(2):
                csb = bfs.tile([128, BL], F32, tag="csb")
                nc.vector.tensor_copy(csb[:], ctx_ps[eh][:])
                nc.sync.dma_start(out=d_ctxo[eh], in_=csb[:])
            for g in range(NG):
                nc.sync.dma_start(out=d_h2o[g], in_=persist["h2f"][g][:])
                nc.sync.dma_start(out=d_s0o[g], in_=persist["s0"][g][:])

    nc.compile()
    return nc


def _prep_host(inputs):
    """Host-side weight preprocessing (shared across cores)."""
    bf = ml_dtypes.bfloat16
    f32 = np.float32
    w1 = np.asarray(inputs["attn_w1"], f32)
    w1hcT = w1[:, :2 * D].T.copy()
    w1hcT[:D] *= 0.5                       # h enters as h2 = 2h
    gs = np.ones((4 * D,), f32)
    gs[2 * D:3 * D] = 2.0                  # g-gate rows: tanh(x/2) trick
    whhT = (np.asarray(inputs["W_hh"], f32) * gs[:, None]).T * 0.5
    wihbT = np.zeros((128, 4 * D), f32)
    wihbT[0] = np.asarray(inputs["W_ih"], f32)[:, 0] * gs
    wihbT[1] = (np.asarray(inputs["b_ih"], f32)
                + np.asarray(inputs["b_hh"], f32)) * gs
    fcw = np.asarray(inputs["fc_w"], f32)
    shared = {
        "w1eT": np.ascontiguousarray(w1[:, 2 * D:].T).astype(bf),
        "w1hcT": np.ascontiguousarray(w1hcT).astype(bf),
        "w2c": np.ascontiguousarray(
            np.asarray(inputs["attn_w2"], f32)[0].reshape(2, 128).T).astype(bf),
        "b1c": np.ascontiguousarray(
            np.asarray(inputs["attn_b1"], f32).reshape(2, 128).T),
        "fcwc": np.ascontiguousarray(fcw[0, :E].reshape(2, 128).T).astype(bf),
        "whhT": np.ascontiguousarray(whhT).astype(bf),
        "wihbT": wihbT.astype(bf),
    }
    return shared


def _postprocess(results, inputs):
    fcf_w = np.asarray(inputs["fcf_w"], np.float32)
    fcf_b = np.asarray(inputs["fcf_b"], np.float32)
    outs = []
    for c in range(NCORES):
        r = results[c]
        h2 = r["h2o"].reshape(NG, 128, 2, GB)          # [g, p, dh, bl]
        h = 0.5 * np.transpose(h2, (0, 3, 2, 1)).reshape(BL, D)
        s0 = r["s0o"].reshape(NG * GB)                 # [g*GB+bl]
        ctx = np.transpose(r["ctxo"], (2, 0, 1)).reshape(BL, E) / s0[:, None]
        outs.append(np.concatenate([h, ctx], axis=1))
    hc = np.concatenate(outs, axis=0)                  # (B, D+E)
    return (hc @ fcf_w.T + fcf_b).astype(np.float32)


def _make_in_maps(inputs):
    shared = _prep_host(inputs)
    enc_full = np.asarray(inputs["input_encoded"], np.float32)
    y_full = np.asarray(inputs["y_history"], np.float32)
    fcw = np.asarray(inputs["fc_w"], np.float32)
    fcb = np.asarray(inputs["fc_b"], np.float32)
    in_maps = []
    for c in range(NCORES):
        sl = slice(c * BL, (c + 1) * BL)
        yterm = (y_full[sl, :, 0].T * fcw[0, E] + fcb[0])   # (T, BL)
        in_maps.append({
            "enc": np.ascontiguousarray(enc_full[sl].reshape(BL * T, E)),
            "yterm": np.ascontiguousarray(yterm.reshape(1, T * BL),
                                          dtype=np.float32),
            **shared,
        })
    return in_maps


def kernel(**inputs):
    global _BUILT
    if _BUILT is None:
        _BUILT = build_nc()
    in_maps = _make_in_maps(inputs)
    res = run_bass_kernel_spmd(_BUILT, in_maps, list(range(NCORES)))
    return _postprocess(res.results, inputs)


def run_traced(inputs, tmpdir=None):
    """Run once with NTFF tracing; returns BassKernelResults."""
    global _BUILT
    if _BUILT is None:
        _BUILT = build_nc()
    in_maps = _make_in_maps(inputs)
    return run_bass_kernel_spmd(_BUILT, in_maps, list(range(NCORES)),
                                trace=True, tmpdir=tmpdir)
